# revision 6
# baseline (speedup 1.0000x reference)
# Trainium2 Bass kernel for nn_BottleNeck (sparse local attention bottleneck).
#
# Sharding: data-parallel over batch (B=8 -> 8 cores, one image each).
# BatchNorm batch-statistics are computed as per-core partials and combined
# with three tiny (1-2KB) AllReduce collectives.
#
# On-chip layout: channels on partitions, hw=32*32=1024 on the free dim.
# Channels are PERMUTED so that partitions 0:63 hold the "x-type" attention
# channels (rel depends only on kh) of all 8 groups and 64:127 the "y-type"
# (rel depends only on kw). The permutation is folded into W1/Wq/Wk/Wv/W3 and
# the BN parameters host-side; conv3 un-permutes, so the output is in the
# original channel order.
#
# v2 structure (vs the earlier fused-STT version):
#  - k-conv bias bk is folded into the rel columns (exact); v-conv bias bv is
#    dropped entirely (BN2 in training mode is invariant to per-channel
#    shifts; att = S'/Z + bv and BN2(att) == BN2(S'/Z) exactly).
#  - The 49 (k_shift + rel) adds run as DVE tensor_scalar_add in 4x perf mode
#    (bf16, packed, SBUF); the *q multiply is ONE batched bf16 tensor_tensor
#    per kh at DVE 2x; exp runs on ACT; e*v is two parity-batched DVE 2x
#    tensor_tensors; Z/S accumulate on the TensorEngine as identity matmuls.
#  - BatchNorm a/b params are computed entirely on the DVE with a
#    bit-trick rsqrt (quake seed + 2 Newton steps), so the ONLY table-based
#    ACT function used anywhere is Exp/Identity/Copy/Relu/Square -- all in
#    one act-table set => no LoadActFuncSet thrash (was 6 loads/rep).
#  - Engine-assignment knobs (env) let individual adds / copies / the final
#    residual ops move between DVE / ACT / GPSIMD(Pool) for load balance.

import os
from contextlib import ExitStack

import numpy as np
import ml_dtypes

import concourse.bass as bass
import concourse.mybir as mybir
import concourse.tile as tile
from concourse import bacc
from concourse.ap import AP
from concourse.bass_utils import run_bass_kernel_spmd

F32 = mybir.dt.float32
BF16 = mybir.dt.bfloat16
I32 = mybir.dt.int32
AF = mybir.ActivationFunctionType
OP = mybir.AluOpType
AX = mybir.AxisListType

B, C_IN, H, W = 8, 512, 32, 32
PLANES, GROUPS, KS, PAD = 128, 8, 7, 3
D = PLANES // GROUPS
REL = D // 2
HW = H * W
PW = W + 2 * PAD            # 38
PHW = PW * PW               # 1444
EPS = 1e-5
N_CORES = 8
NSAMP = float(B * HW)       # batchnorm sample count over (N,H,W)

# kw plane order inside the per-kh buffers: evens first, then odds, so that
# one AP (kw step 2) covers each parity block contiguously.
KW_ORDER = [0, 2, 4, 6, 1, 3, 5]

# --------- engine-assignment knobs (tuning) ---------
def _kwset(name, default):
    v = os.environ.get(name, default)
    return {int(x) for x in v.split(",") if x != ""}

ACT_ADD_KWS = _kwset("BASS_ADD_A", "")      # kw planes whose +rel runs on ACT
POOL_ADD_KWS = _kwset("BASS_ADD_P", "")     # ... on GPSIMD/Pool
_ATT_ENG = os.environ.get("BASS_ATT_ENG", "v")    # att = S*(1/Z): v=DVE p=Pool
_O3_ENG = os.environ.get("BASS_O3_ENG", "a")      # o3 psum->sbuf copy+sum
_FIN_RELU = os.environ.get("BASS_FIN_RELU", "p")  # final relu: a=ACT p=Pool
_FIN_STT = os.environ.get("BASS_FIN_STT", "v")    # final a3*o3+xf: v=DVE p=Pool
_EV_POOL_KWS = _kwset("BASS_EV_P", "")      # ev planes (by pos) on Pool
_CC_MODE = os.environ.get("BASS_CC_MODE", "ar")
_NO_CC = os.environ.get("BASS_NO_CC") == "1"
_REPS = int(os.environ.get("BASS_REPS", "1"))
_NO_ATT = os.environ.get("BASS_NO_ATT") == "1"

_RSQRT_K = 0x5F3759DF


def _sview(flat_ap, off, dims):
    """Hand-built strided view of an SBUF tile ([partition] + dims)."""
    return AP(flat_ap.tensor, off, [list(flat_ap.ap[0])] + [list(d) for d in dims])


def _build_nc():
    nc = bacc.Bacc("TRN2", target_bir_lowering=False, debug=False,
                   num_devices=N_CORES)

    xf_d = nc.dram_tensor("xf", [C_IN, HW], F32, kind="ExternalInput")
    xb_d = nc.dram_tensor("xb", [C_IN, HW], BF16, kind="ExternalInput")
    w1t_d = nc.dram_tensor("w1t", [C_IN, PLANES], BF16, kind="ExternalInput")
    mqkv_d = nc.dram_tensor("mqkv", [3, PLANES, PLANES], BF16, kind="ExternalInput")
    w3t_d = nc.dram_tensor("w3t", [PLANES, 4 * PLANES], BF16, kind="ExternalInput")
    relc_d = nc.dram_tensor("relc", [PLANES, KS * KS], F32, kind="ExternalInput")
    gb12_d = nc.dram_tensor("gb12", [PLANES, 4], F32, kind="ExternalInput")
    gb3_d = nc.dram_tensor("gb3", [PLANES, 8], F32, kind="ExternalInput")
    id_d = nc.dram_tensor("id128", [PLANES, PLANES], BF16, kind="ExternalInput")
    out_d = nc.dram_tensor("out", [C_IN, HW], F32, kind="ExternalOutput")

    dbg = os.environ.get("BASS_KDBG") == "1"
    if dbg:
        dbg_d = {n: nc.dram_tensor(f"dbg_{n}", shp, F32, kind="ExternalOutput")
                 for n, shp in [("o1", [128, HW]), ("x1p", [128, PHW]),
                                ("q", [128, HW]), ("kf", [128, PHW]),
                                ("eb0", [128, 7 * HW]), ("z", [128, HW]),
                                ("s", [128, HW]), ("att", [128, HW]),
                                ("o3", [128, 4 * HW]), ("a1", [128, 1]),
                                ("b1", [128, 1])]}

    with tile.TileContext(nc) as tc, ExitStack() as ctx:
        const = ctx.enter_context(tc.tile_pool(name="const", bufs=1))
        sb = ctx.enter_context(tc.tile_pool(name="sb", bufs=1))
        work = ctx.enter_context(tc.tile_pool(name="work", bufs=2))
        psum = ctx.enter_context(tc.tile_pool(name="psum", bufs=1, space="PSUM"))
        dram = ctx.enter_context(tc.tile_pool(name="dram", bufs=1, space="DRAM"))

        # ---------------- constants / weights ----------------
        id_sb = const.tile([128, 128], BF16)
        nc.sync.dma_start(id_sb[:], id_d[:])
        w1t_sb = const.tile([128, 4, 128], BF16)
        for k in range(4):
            nc.sync.dma_start(w1t_sb[:, k, :], w1t_d[k * 128:(k + 1) * 128, :])
        mqkv_sb = const.tile([128, 3, 128], BF16)
        for i in range(3):
            nc.sync.dma_start(mqkv_sb[:, i, :], mqkv_d[i])
        w3t_sb = const.tile([128, 512], BF16)
        nc.sync.dma_start(w3t_sb[:], w3t_d[:])
        relc_sb = const.tile([128, 49], F32)
        nc.sync.dma_start(relc_sb[:], relc_d[:])
        gb12_sb = const.tile([128, 4], F32)
        nc.sync.dma_start(gb12_sb[:], gb12_d[:])
        gb3_sb = const.tile([128, 8], F32)
        nc.sync.dma_start(gb3_sb[:], gb3_d[:])

        zcol = const.tile([128, 1], F32)
        nc.gpsimd.memset(zcol[:], 0.0)
        expwarm = const.tile([128, 1], F32)
        nc.scalar.activation(expwarm[:], zcol[:], AF.Exp, bias=zcol[:])

        # x1 padded field: border is zero and only the interior is rewritten
        # each rep, so clear it once.
        x1p = const.tile([128, PW, PW], BF16)
        nc.gpsimd.memset(x1p[:], 0.0)

        # ---------------- helpers ----------------
        def dump(name, ap):
            if not dbg:
                return
            n = ap.free_size()
            scr = work.tile([128, 7 * HW], F32, tag="dbgscr", bufs=1,
                            name=f"dbgscr_{name}")[:, 0:n]
            nc.vector.tensor_copy(scr[:], ap)
            nc.sync.dma_start(dbg_d[name][:], scr[:])

        def allreduce(src_ap, ncols, name):
            dst = sb.tile([128, ncols], F32, name=f"cc_{name}_res")
            if _NO_CC:
                nc.vector.tensor_scalar_mul(dst[:], src_ap, float(N_CORES))
                return dst
            cin = dram.tile([128, ncols], F32, name=f"cc_{name}_in")
            nc.sync.dma_start(cin[:], src_ap)
            if _CC_MODE == "ag":
                cout = dram.tile([N_CORES * 128, ncols], F32,
                                 name=f"cc_{name}_out")
                nc.gpsimd.collective_compute(
                    "AllGather", OP.bypass,
                    replica_groups=[list(range(N_CORES))],
                    ins=[cin[:].opt()], outs=[cout[:].opt()],
                )
                gat = sb.tile([128, ncols, N_CORES], F32, name=f"cc_{name}_gat")
                nc.sync.dma_start(
                    gat[:], cout[:].rearrange("(j p) c -> p c j", p=128))
                nc.vector.tensor_reduce(dst[:], gat[:], AX.X, OP.add)
            else:
                cout = dram.tile([128, ncols], F32, name=f"cc_{name}_out")
                nc.gpsimd.collective_compute(
                    "AllReduce", OP.add,
                    replica_groups=[list(range(N_CORES))],
                    ins=[cin[:].opt()], outs=[cout[:].opt()],
                )
                nc.sync.dma_start(dst[:], cout[:])
            return dst

        def rsqrt_dve(y_f32, x_ap, scr_i32, n, name):
            """y = 1/sqrt(x) on DVE only: quake seed + 2 Newton steps.

            y_f32/scr_i32: [128,n] f32/i32 compact tiles; x_ap compact f32."""
            xi = x_ap.bitcast(I32)
            nc.vector.tensor_scalar(scr_i32[:], xi, 1, None,
                                    OP.logical_shift_right)
            nc.vector.tensor_scalar(scr_i32[:], scr_i32[:], -1, None,
                                    OP.bitwise_xor)
            nc.vector.tensor_scalar(scr_i32[:], scr_i32[:], _RSQRT_K + 1, None,
                                    OP.add)
            y0 = scr_i32[:].bitcast(F32)
            t = sb.tile([128, n], F32, name=f"rs_{name}_t")
            for it in range(2):
                src = y0 if it == 0 else y_f32[:]
                nc.vector.tensor_tensor(t[:], src, src, OP.mult)
                nc.vector.scalar_tensor_tensor(t[:], t[:], 0.5, x_ap,
                                               OP.mult, OP.mult)
                nc.vector.tensor_scalar(t[:], t[:], -1.0, 1.5, OP.mult, OP.add)
                nc.vector.tensor_tensor(y_f32[:], src, t[:], OP.mult)

        def bn_params(gsum, gsumsq, gamma, beta, name, n=1):
            """a = gamma*rsqrt(var+eps), b = beta - a*mean; all on DVE."""
            mean = sb.tile([128, n], F32, name=f"bn_{name}_mean")
            nc.vector.tensor_scalar_mul(mean[:], gsum, 1.0 / NSAMP)
            msq = sb.tile([128, n], F32, name=f"bn_{name}_msq")
            nc.vector.tensor_scalar(msq[:], gsumsq, 1.0 / NSAMP, EPS,
                                    OP.mult, OP.add)
            var = sb.tile([128, n], F32, name=f"bn_{name}_var")
            nc.vector.tensor_tensor(var[:], mean[:], mean[:], OP.mult)
            nc.vector.tensor_tensor(var[:], msq[:], var[:], OP.subtract)
            rsd = sb.tile([128, n], F32, name=f"bn_{name}_rsd")
            scr = sb.tile([128, n], I32, name=f"bn_{name}_scr")
            rsqrt_dve(rsd, var[:], scr, n, name)
            a = sb.tile([128, n], F32, name=f"bn_{name}_a")
            b = sb.tile([128, n], F32, name=f"bn_{name}_b")
            nc.vector.tensor_tensor(a[:], gamma, rsd[:], OP.mult)
            nc.vector.tensor_tensor(b[:], a[:], mean[:], OP.mult)
            nc.vector.tensor_tensor(b[:], beta, b[:], OP.subtract)
            return a, b

        for _rep in range(_REPS):
            xb_sb = sb.tile([128, 4, HW], BF16)
            for k in range(4):
                for hh in range(2):
                    nc.sync.dma_start(
                        xb_sb[:, k, hh * 512:(hh + 1) * 512],
                        xb_d[k * 128:(k + 1) * 128, hh * 512:(hh + 1) * 512])

            # ---------------- conv1 (512 -> 128) ----------------
            o1_ps = psum.tile([128, HW], F32, tag="A")
            for n in range(2):
                for k in range(4):
                    nc.tensor.matmul(
                        o1_ps[:, n * 512:(n + 1) * 512],
                        w1t_sb[:, k, :],
                        xb_sb[:, k, n * 512:(n + 1) * 512],
                        start=(k == 0), stop=(k == 3),
                    )

            s1 = sb.tile([128, 2], F32)
            nc.vector.tensor_reduce(s1[:, 0:1], o1_ps[:], AX.X, OP.add)
            sq1 = work.tile([128, HW], BF16, tag="sq", bufs=1)
            nc.scalar.activation(sq1[:], o1_ps[:], AF.Square, bias=zcol[:],
                                 accum_out=s1[:, 1:2])
            dump("o1", o1_ps[:])
            g1 = allreduce(s1[:], 2, "bn1")
            a1, b1 = bn_params(g1[:, 0:1], g1[:, 1:2],
                               gb12_sb[:, 0:1], gb12_sb[:, 1:2], "1")
            dump("a1", a1[:])
            dump("b1", b1[:])

            # x1 = relu(a1*o1 + b1), written into the zero-padded 38x38 field
            nc.scalar.activation(
                x1p[:, PAD:PAD + H, PAD:PAD + W],
                o1_ps[:].rearrange("p (y x) -> p y x", y=H),
                AF.Relu, bias=b1[:], scale=a1[:],
            )

            dump("x1p", x1p[:].rearrange("p y x -> p (y x)"))
            # ---------------- q/k/v grouped 1x1 convs ----------------

            col_splits = [(0, 512), (512, 1024), (1024, PHW)]

            def kv_conv(widx, name, ps_tag):
                ps = psum.tile([128, PHW], F32, tag=ps_tag, name=f"{name}_ps")
                x1p_flat = x1p[:].rearrange("p y x -> p (y x)")
                for (c0, c1) in col_splits:
                    nc.tensor.matmul(
                        ps[:, c0:c1],
                        mqkv_sb[:, widx, :],
                        x1p_flat[:, c0:c1],
                        start=True, stop=True,
                    )
                fld = sb.tile([128, PW, PW], BF16, name=f"{name}_fld")
                nc.scalar.activation(
                    fld[:], ps[:].rearrange("p (y x) -> p y x", y=PW),
                    AF.Copy,
                )
                odd = sb.tile([128, PHW - 1], BF16, name=f"{name}_odd")
                nc.vector.tensor_copy(
                    odd[:], fld[:].rearrange("p y x -> p (y x)")[:, 1:PHW])
                return fld, odd

            q_ps = psum.tile([128, HW], F32, tag="A")
            for n in range(2):
                nc.tensor.matmul(
                    q_ps[:, n * 512:(n + 1) * 512],
                    mqkv_sb[:, 0, :],
                    x1p[:, PAD + n * 16:PAD + (n + 1) * 16, PAD:PAD + W],
                    start=True, stop=True,
                )
            q_bf = sb.tile([128, HW], BF16)
            nc.scalar.activation(q_bf[:], q_ps[:], AF.Copy)
            dump("q", q_bf[:])
            k_fld, k_odd = kv_conv(1, "k", "B")
            dump("kf", k_fld[:].rearrange("p y x -> p (y x)"))
            v_fld, v_odd = kv_conv(2, "v", "C")

            # ---------------- attention over 49 shifts ----------------
            z_ps = psum.tile([128, HW], F32, tag="A")
            s_ps = psum.tile([128, HW], F32, tag="B")

            q_b7 = q_bf[:].rearrange("p (a h) -> p a h", a=1).to_broadcast(
                [128, KS, HW])

            # Software-pipelined: the e*v multiply and S accumulation of
            # iteration kh-1 are emitted AFTER exp(kh) is issued, so the DVE
            # never stalls waiting on the ACT exp it just fed.
            def emit_ev(kh, eb):
                evb = work.tile([128, KS, HW], BF16, tag="evb",
                                name=f"evb_{kh}")
                v_flat = v_fld[:].rearrange("p y x -> p (y x)")
                ev_even = _sview(v_flat, kh * PW, [(2, 4), (PW, H), (1, W)])
                ev_odd = _sview(v_odd[:], kh * PW, [(2, 3), (PW, H), (1, W)])
                pool_pos = sorted(_EV_POOL_KWS)
                nc.vector.tensor_tensor(
                    evb[:, 0:4, :].rearrange("p k (y x) -> p k y x", y=H),
                    eb[:, 0:4, :].rearrange("p k (y x) -> p k y x", y=H),
                    ev_even, OP.mult)
                if 6 in pool_pos:
                    nc.vector.tensor_tensor(
                        evb[:, 4:6, :].rearrange("p k (y x) -> p k y x", y=H),
                        eb[:, 4:6, :].rearrange("p k (y x) -> p k y x", y=H),
                        _sview(v_odd[:], kh * PW, [(2, 2), (PW, H), (1, W)]),
                        OP.mult)
                    nc.gpsimd.tensor_tensor(
                        evb[:, 6, :].rearrange("p (y x) -> p y x", y=H),
                        eb[:, 6, :].rearrange("p (y x) -> p y x", y=H),
                        _sview(v_odd[:], kh * PW + 4, [(PW, H), (1, W)]),
                        OP.mult)
                else:
                    nc.vector.tensor_tensor(
                        evb[:, 4:7, :].rearrange("p k (y x) -> p k y x", y=H),
                        eb[:, 4:7, :].rearrange("p k (y x) -> p k y x", y=H),
                        ev_odd, OP.mult)
                # S += sum_planes(ev)
                for pl in range(KS):
                    for hh in range(2):
                        nc.tensor.matmul(
                            s_ps[:, hh * 512:(hh + 1) * 512],
                            id_sb[:], evb[:, pl, hh * 512:(hh + 1) * 512],
                            start=(kh == 0 and pl == 0),
                            stop=(kh == KS - 1 and pl == KS - 1),
                            skip_group_check=True,
                        )

            prev = None
            for kh in range([0, KS][not _NO_ATT]):
                mb = work.tile([128, KS, HW], BF16, tag="mb")
                # m = k_shift + (rel + bk); rel is a per-partition scalar
                for pos, kw in enumerate(KW_ORDER):
                    kap = kh * KS + kw
                    if kw % 2 == 0:
                        view = k_fld[:, kh:kh + H, kw:kw + W]
                    else:
                        view = _sview(k_odd[:], kh * PW + kw - 1,
                                      [(PW, H), (1, W)])
                    dst = mb[:, pos, :].rearrange("p (y x) -> p y x", y=H)
                    if kw in ACT_ADD_KWS:
                        nc.scalar.activation(dst, view, AF.Identity,
                                             bias=relc_sb[:, kap:kap + 1])
                    elif kw in POOL_ADD_KWS:
                        nc.gpsimd.tensor_scalar_add(dst, view,
                                                    relc_sb[:, kap:kap + 1])
                    else:
                        nc.vector.tensor_scalar_add(dst, view,
                                                    relc_sb[:, kap:kap + 1])
                # l = m * q  (one batched bf16 2x tensor_tensor)
                nc.vector.tensor_tensor(mb[:], mb[:], q_b7, OP.mult)
                # e = exp(l)
                eb = work.tile([128, KS, HW], BF16, tag="eb", bufs=3,
                               name=f"eb_{kh}")
                nc.scalar.activation(eb[:], mb[:], AF.Exp, bias=zcol[:])
                # Z += sum_planes(e) on the TensorEngine
                for pl in range(KS):
                    for hh in range(2):
                        nc.tensor.matmul(
                            z_ps[:, hh * 512:(hh + 1) * 512],
                            id_sb[:], eb[:, pl, hh * 512:(hh + 1) * 512],
                            start=(kh == 0 and pl == 0),
                            stop=(kh == KS - 1 and pl == KS - 1),
                            skip_group_check=True,
                        )
                if kh == 0:
                    dump("eb0", eb[:].rearrange("p k h -> p (k h)"))
                if prev is not None:
                    emit_ev(*prev)
                prev = (kh, eb)
            if prev is not None:
                emit_ev(*prev)

            # att = S / Z, then BN2 + relu
            dump("z", z_ps[:])
            dump("s", s_ps[:])
            rz = sb.tile([128, HW], F32)
            nc.vector.reciprocal_approx_fast(rz[:], z_ps[:])
            att = sb.tile([128, HW], F32)
            if _ATT_ENG == "p":
                nc.gpsimd.tensor_tensor(att[:], s_ps[:], rz[:], OP.mult)
            else:
                nc.vector.tensor_tensor(att[:], s_ps[:], rz[:], OP.mult)

            dump("att", att[:])
            s2 = sb.tile([128, 2], F32)
            nc.vector.tensor_reduce(s2[:, 0:1], att[:], AX.X, OP.add)
            sq2 = work.tile([128, HW], BF16, tag="sq", bufs=1)
            nc.scalar.activation(sq2[:], att[:], AF.Square, bias=zcol[:],
                                 accum_out=s2[:, 1:2])
            g2 = allreduce(s2[:], 2, "bn2")
            a2, b2 = bn_params(g2[:, 0:1], g2[:, 1:2],
                               gb12_sb[:, 2:3], gb12_sb[:, 3:4], "2")
            x2 = sb.tile([128, HW], BF16)
            nc.scalar.activation(x2[:], att[:], AF.Relu, bias=b2[:], scale=a2[:])

            # ---------------- conv3 (128 -> 512) + BN3 + residual ----------------
            xf_sb = sb.tile([128, 4, HW], F32)
            for k in range(4):
                nc.sync.dma_start(xf_sb[:, k, :], xf_d[k * 128:(k + 1) * 128, :])
            o3_sb = sb.tile([128, 4, HW], F32)
            s3 = sb.tile([128, 8], F32)
            for j in range(4):
                o3_ps = psum.tile([128, HW], F32, tag=["C", "B"][j % 2],
                                  name=f"o3_ps_{j}")
                for n in range(2):
                    nc.tensor.matmul(
                        o3_ps[:, n * 512:(n + 1) * 512],
                        w3t_sb[:, j * 128:(j + 1) * 128],
                        x2[:, n * 512:(n + 1) * 512],
                        start=True, stop=True,
                    )
                sq3 = work.tile([128, HW], BF16, tag="sq", bufs=1,
                                name=f"sq3_{j}")
                nc.scalar.activation(sq3[:], o3_ps[:], AF.Square, bias=zcol[:],
                                     accum_out=s3[:, 2 * j + 1:2 * j + 2])
                if _O3_ENG == "p":
                    nc.gpsimd.tensor_scalar(o3_sb[:, j, :], o3_ps[:], 0.0, None,
                                            OP.add,
                                            accum_out=s3[:, 2 * j:2 * j + 1])
                else:
                    nc.scalar.activation(o3_sb[:, j, :], o3_ps[:], AF.Copy,
                                         accum_out=s3[:, 2 * j:2 * j + 1])

            dump("o3", o3_sb[:].rearrange("p j h -> p (j h)"))
            g3 = allreduce(s3[:], 8, "bn3")
            # batched BN3 params for all 4 chunks (compact copies first so the
            # rsqrt bitcast sees contiguous columns)
            gsum3 = sb.tile([128, 4], F32)
            nc.vector.tensor_copy(gsum3[:], g3[:, 0:8:2])
            gsq3 = sb.tile([128, 4], F32)
            nc.vector.tensor_copy(gsq3[:], g3[:, 1:8:2])
            a3, b3 = bn_params(gsum3[:], gsq3[:],
                               gb3_sb[:, 0:8:2], gb3_sb[:, 1:8:2], "3", n=4)
            for j in range(4):
                t3 = work.tile([128, HW], F32, tag="t3", name=f"t3_{j}")
                if _FIN_STT == "p":
                    nc.gpsimd.scalar_tensor_tensor(
                        t3[:], o3_sb[:, j, :], a3[:, j:j + 1], xf_sb[:, j, :],
                        OP.mult, OP.add)
                else:
                    nc.vector.scalar_tensor_tensor(
                        t3[:], o3_sb[:, j, :], a3[:, j:j + 1], xf_sb[:, j, :],
                        OP.mult, OP.add)
                ot = work.tile([128, HW], F32, tag="ot", name=f"ot_{j}")
                if _FIN_RELU == "p":
                    nc.gpsimd.tensor_scalar(ot[:], t3[:], b3[:, j:j + 1], 0.0,
                                            OP.add, OP.max)
                else:
                    nc.scalar.activation(ot[:], t3[:], AF.Relu,
                                         bias=b3[:, j:j + 1])
                nc.sync.dma_start(out_d[j * 128:(j + 1) * 128, :], ot[:])

    nc.compile()
    return nc


_NC = None


def _get_nc():
    global _NC
    if _NC is None:
        _NC = _build_nc()
    return _NC


def _prep_inputs(x, W1, g1, b1, Wq, Wk, bk, Wv, bv, rel_x, rel_y, g2, b2, W3, g3, b3):
    f32 = np.float32
    bf = ml_dtypes.bfloat16

    # channel permutation: new partition -> old channel within the 128 planes
    perm = np.zeros(PLANES, dtype=np.int64)
    for g in range(GROUPS):
        for d in range(D):
            p = g * REL + d if d < REL else 64 + g * REL + (d - REL)
            perm[p] = g * D + d

    W1p = np.ascontiguousarray(W1[perm, :])                     # [128, 512]
    w1t = np.ascontiguousarray(W1p.T).astype(bf)                # [512, 128]

    def block_mat(Wg):
        M = np.zeros((PLANES, PLANES), dtype=f32)
        for po in range(PLANES):
            g = (po % 64) // REL
            o = perm[po] - g * D
            for pi_d in range(D):
                pi = g * REL + pi_d if pi_d < REL else 64 + g * REL + (pi_d - REL)
                M[po, pi] = Wg[g, o, pi_d]
        return M

    mqkv = np.stack([np.ascontiguousarray(block_mat(Wg).T)
                     for Wg in (Wq, Wk, Wv)]).astype(bf)        # [3,128,128] (lhsT)

    # rel columns [128, 49]; k-conv bias bk is folded in (exact: the adds
    # apply rel+bk to every window position, matching conv-bias semantics
    # including the zero-padding border). v-conv bias bv is dropped: BN2 is
    # invariant to the per-channel shift it would produce.
    bkp = bk.reshape(-1)[perm].astype(f32)
    relc = np.zeros((PLANES, KS * KS), dtype=f32)
    for p in range(PLANES):
        g = (p % 64) // REL
        dd = perm[p] - g * D
        for kap in range(KS * KS):
            kh, kw = divmod(kap, KS)
            base = rel_x[dd, kh, 0] if dd < REL else rel_y[dd - REL, 0, kw]
            relc[p, kap] = base + bkp[p]

    gb12 = np.stack([g1[perm], b1[perm], g2[perm], b2[perm]], axis=1).astype(f32)

    W3p = np.ascontiguousarray(W3[:, perm])                     # [512, 128]
    w3t = np.ascontiguousarray(W3p.T).astype(bf)                # [128, 512]

    gb3 = np.zeros((PLANES, 8), dtype=f32)
    for j in range(4):
        gb3[:, 2 * j] = g3[j * 128:(j + 1) * 128]
        gb3[:, 2 * j + 1] = b3[j * 128:(j + 1) * 128]

    id128 = np.eye(PLANES, dtype=f32).astype(bf)

    shared = dict(w1t=w1t, mqkv=mqkv, w3t=w3t, relc=relc,
                  gb12=gb12, gb3=gb3, id128=id128)
    in_maps = []
    for c in range(N_CORES):
        xi = np.ascontiguousarray(x[c].reshape(C_IN, HW)).astype(f32)
        m = dict(shared)
        m["xf"] = xi
        m["xb"] = xi.astype(bf)
        in_maps.append(m)
    return in_maps


def _run(inputs, **kw):
    nc = _get_nc()
    in_maps = _prep_inputs(**inputs)
    res = run_bass_kernel_spmd(nc, in_maps, core_ids=list(range(N_CORES)), **kw)
    out = np.stack([res.results[c]["out"].reshape(C_IN, H, W)
                    for c in range(N_CORES)]).astype(np.float32)
    return out, res


def kernel(**inputs):
    out, _ = _run(inputs)
    return out


# revision 8
# speedup vs baseline: 1.1796x; 1.1796x over previous
# Trainium2 Bass kernel for nn_BottleNeck (sparse local attention bottleneck).
#
# Sharding: data-parallel over batch (B=8 -> 8 cores, one image each).
# BatchNorm batch-statistics are computed as per-core partials and combined
# with three tiny (1-2KB) AllReduce collectives.
#
# On-chip layout: channels on partitions, hw=32*32=1024 on the free dim.
# Channels are PERMUTED so that partitions 0:63 hold the "x-type" attention
# channels (rel depends only on kh) of all 8 groups and 64:127 the "y-type"
# (rel depends only on kw). The permutation is folded into W1/Wq/Wk/Wv/W3 and
# the BN parameters host-side; conv3 un-permutes, so the output is in the
# original channel order.
#
# v2 structure (vs the earlier fused-STT version):
#  - k-conv bias bk is folded into the rel columns (exact); v-conv bias bv is
#    dropped entirely (BN2 in training mode is invariant to per-channel
#    shifts; att = S'/Z + bv and BN2(att) == BN2(S'/Z) exactly).
#  - The 49 (k_shift + rel) adds run as DVE tensor_scalar_add in 4x perf mode
#    (bf16, packed, SBUF); the *q multiply is ONE batched bf16 tensor_tensor
#    per kh at DVE 2x; exp runs on ACT; e*v is two parity-batched DVE 2x
#    tensor_tensors; Z/S accumulate on the TensorEngine as identity matmuls.
#  - BatchNorm a/b params are computed entirely on the DVE with a
#    bit-trick rsqrt (quake seed + 2 Newton steps), so the ONLY table-based
#    ACT function used anywhere is Exp/Identity/Copy/Relu/Square -- all in
#    one act-table set => no LoadActFuncSet thrash (was 6 loads/rep).
#  - Engine-assignment knobs (env) let individual adds / copies / the final
#    residual ops move between DVE / ACT / GPSIMD(Pool) for load balance.

import os
from contextlib import ExitStack

import numpy as np
import ml_dtypes

import concourse.bass as bass
import concourse.mybir as mybir
import concourse.tile as tile
from concourse import bacc
from concourse.ap import AP
from concourse.bass_utils import run_bass_kernel_spmd

F32 = mybir.dt.float32
BF16 = mybir.dt.bfloat16
I32 = mybir.dt.int32
AF = mybir.ActivationFunctionType
OP = mybir.AluOpType
AX = mybir.AxisListType

B, C_IN, H, W = 8, 512, 32, 32
PLANES, GROUPS, KS, PAD = 128, 8, 7, 3
D = PLANES // GROUPS
REL = D // 2
HW = H * W
PW = W + 2 * PAD            # 38
PHW = PW * PW               # 1444
EPS = 1e-5
N_CORES = 8
NSAMP = float(B * HW)       # batchnorm sample count over (N,H,W)

# kw plane order inside the per-kh buffers: evens first, then odds, so that
# one AP (kw step 2) covers each parity block contiguously.
KW_ORDER = [0, 2, 4, 6, 1, 3, 5]

# --------- engine-assignment knobs (tuning) ---------
def _kwset(name, default):
    v = os.environ.get(name, default)
    return {int(x) for x in v.split(",") if x != ""}

ACT_ADD_KWS = _kwset("BASS_ADD_A", "")      # kw planes whose +rel runs on ACT
POOL_ADD_KWS = _kwset("BASS_ADD_P", "")     # ... on GPSIMD/Pool
_ATT_ENG = os.environ.get("BASS_ATT_ENG", "v")    # att = S*(1/Z): v=DVE p=Pool
_O3_ENG = os.environ.get("BASS_O3_ENG", "a")      # o3 psum->sbuf copy+sum
_FIN_RELU = os.environ.get("BASS_FIN_RELU", "p")  # final relu: a=ACT p=Pool
_FIN_STT = os.environ.get("BASS_FIN_STT", "v")    # final a3*o3+xf: v=DVE p=Pool
_EV_POOL_KWS = _kwset("BASS_EV_P", "")      # ev planes (by pos) on Pool
_CC_MODE = os.environ.get("BASS_CC_MODE", "ar")
_NO_CC = os.environ.get("BASS_NO_CC") == "1"
_REPS = int(os.environ.get("BASS_REPS", "1"))
_NO_ATT = os.environ.get("BASS_NO_ATT") == "1"

_RSQRT_K = 0x5F3759DF


def _sview(flat_ap, off, dims):
    """Hand-built strided view of an SBUF tile ([partition] + dims)."""
    return AP(flat_ap.tensor, off, [list(flat_ap.ap[0])] + [list(d) for d in dims])


def _build_nc():
    nc = bacc.Bacc("TRN2", target_bir_lowering=False, debug=False,
                   num_devices=N_CORES)

    xf_d = nc.dram_tensor("xf", [C_IN, HW], F32, kind="ExternalInput")
    xb_d = nc.dram_tensor("xb", [C_IN, HW], BF16, kind="ExternalInput")
    w1t_d = nc.dram_tensor("w1t", [C_IN, PLANES], BF16, kind="ExternalInput")
    mqkv_d = nc.dram_tensor("mqkv", [3, PLANES, PLANES], BF16, kind="ExternalInput")
    w3t_d = nc.dram_tensor("w3t", [PLANES, 4 * PLANES], BF16, kind="ExternalInput")
    relc_d = nc.dram_tensor("relc", [PLANES, KS * KS], F32, kind="ExternalInput")
    gb12_d = nc.dram_tensor("gb12", [PLANES, 4], F32, kind="ExternalInput")
    gb3_d = nc.dram_tensor("gb3", [PLANES, 8], F32, kind="ExternalInput")
    id_d = nc.dram_tensor("id128", [PLANES, PLANES], BF16, kind="ExternalInput")
    out_d = nc.dram_tensor("out", [C_IN, HW], F32, kind="ExternalOutput")

    dbg = os.environ.get("BASS_KDBG") == "1"
    if dbg:
        dbg_d = {n: nc.dram_tensor(f"dbg_{n}", shp, F32, kind="ExternalOutput")
                 for n, shp in [("o1", [128, HW]), ("x1p", [128, PHW]),
                                ("q", [128, HW]), ("kf", [128, PHW]),
                                ("eb0", [128, 7 * HW]), ("z", [128, HW]),
                                ("s", [128, HW]), ("att", [128, HW]),
                                ("o3", [128, 4 * HW]), ("a1", [128, 1]),
                                ("b1", [128, 1])]}

    with tile.TileContext(nc) as tc, ExitStack() as ctx:
        const = ctx.enter_context(tc.tile_pool(name="const", bufs=1))
        sb = ctx.enter_context(tc.tile_pool(name="sb", bufs=1))
        work = ctx.enter_context(tc.tile_pool(name="work", bufs=2))
        psum = ctx.enter_context(tc.tile_pool(name="psum", bufs=1, space="PSUM"))
        dram = ctx.enter_context(tc.tile_pool(name="dram", bufs=1, space="DRAM"))

        # ---------------- constants / weights ----------------
        id_sb = const.tile([128, 128], BF16)
        nc.sync.dma_start(id_sb[:], id_d[:])
        w1t_sb = const.tile([128, 4, 128], BF16)
        for k in range(4):
            nc.sync.dma_start(w1t_sb[:, k, :], w1t_d[k * 128:(k + 1) * 128, :])
        mqkv_sb = const.tile([128, 3, 128], BF16)
        for i in range(3):
            nc.sync.dma_start(mqkv_sb[:, i, :], mqkv_d[i])
        w3t_sb = const.tile([128, 512], BF16)
        nc.sync.dma_start(w3t_sb[:], w3t_d[:])
        relc_sb = const.tile([128, 49], F32)
        nc.sync.dma_start(relc_sb[:], relc_d[:])
        gb12_sb = const.tile([128, 4], F32)
        nc.sync.dma_start(gb12_sb[:], gb12_d[:])
        gb3_sb = const.tile([128, 8], F32)
        nc.sync.dma_start(gb3_sb[:], gb3_d[:])

        zcol = const.tile([128, 1], F32)
        nc.gpsimd.memset(zcol[:], 0.0)
        expwarm = const.tile([128, 1], F32)
        nc.scalar.activation(expwarm[:], zcol[:], AF.Exp, bias=zcol[:])

        # x1 padded field: border is zero and only the interior is rewritten
        # each rep, so clear it once.
        x1p = const.tile([128, PW, PW], BF16)
        nc.gpsimd.memset(x1p[:], 0.0)

        # ---------------- helpers ----------------
        def dump(name, ap):
            if not dbg:
                return
            n = ap.free_size()
            scr = work.tile([128, 7 * HW], F32, tag="dbgscr", bufs=1,
                            name=f"dbgscr_{name}")[:, 0:n]
            nc.vector.tensor_copy(scr[:], ap)
            nc.sync.dma_start(dbg_d[name][:], scr[:])

        def allreduce(src_ap, ncols, name):
            dst = sb.tile([128, ncols], F32, name=f"cc_{name}_res")
            if _NO_CC:
                nc.vector.tensor_scalar_mul(dst[:], src_ap, float(N_CORES))
                return dst
            cin = dram.tile([128, ncols], F32, name=f"cc_{name}_in")
            nc.sync.dma_start(cin[:], src_ap)
            if _CC_MODE == "ag":
                cout = dram.tile([N_CORES * 128, ncols], F32,
                                 name=f"cc_{name}_out")
                nc.gpsimd.collective_compute(
                    "AllGather", OP.bypass,
                    replica_groups=[list(range(N_CORES))],
                    ins=[cin[:].opt()], outs=[cout[:].opt()],
                )
                gat = sb.tile([128, ncols, N_CORES], F32, name=f"cc_{name}_gat")
                nc.sync.dma_start(
                    gat[:], cout[:].rearrange("(j p) c -> p c j", p=128))
                nc.vector.tensor_reduce(dst[:], gat[:], AX.X, OP.add)
            else:
                cout = dram.tile([128, ncols], F32, name=f"cc_{name}_out")
                nc.gpsimd.collective_compute(
                    "AllReduce", OP.add,
                    replica_groups=[list(range(N_CORES))],
                    ins=[cin[:].opt()], outs=[cout[:].opt()],
                )
                nc.sync.dma_start(dst[:], cout[:])
            return dst

        def rsqrt_dve(y_f32, x_ap, scr_i32, n, name):
            """y = 1/sqrt(x) on DVE only: quake seed + 2 Newton steps.

            y_f32/scr_i32: [128,n] f32/i32 compact tiles; x_ap compact f32."""
            xi = x_ap.bitcast(I32)
            nc.vector.tensor_scalar(scr_i32[:], xi, 1, None,
                                    OP.logical_shift_right)
            nc.vector.tensor_scalar(scr_i32[:], scr_i32[:], -1, None,
                                    OP.bitwise_xor)
            nc.vector.tensor_scalar(scr_i32[:], scr_i32[:], _RSQRT_K + 1, None,
                                    OP.add)
            y0 = scr_i32[:].bitcast(F32)
            t = sb.tile([128, n], F32, name=f"rs_{name}_t")
            for it in range(2):
                src = y0 if it == 0 else y_f32[:]
                nc.vector.tensor_tensor(t[:], src, src, OP.mult)
                nc.vector.scalar_tensor_tensor(t[:], t[:], 0.5, x_ap,
                                               OP.mult, OP.mult)
                nc.vector.tensor_scalar(t[:], t[:], -1.0, 1.5, OP.mult, OP.add)
                nc.vector.tensor_tensor(y_f32[:], src, t[:], OP.mult)

        epscol = const.tile([128, 1], F32)
        nc.gpsimd.memset(epscol[:], EPS)

        def bn_params(gsum, gsumsq, gamma, beta, name, n=1):
            """a = gamma*rsqrt(var+eps), b = beta - a*mean; all on DVE."""
            mean = sb.tile([128, n], F32, name=f"bn_{name}_mean")
            nc.vector.tensor_scalar_mul(mean[:], gsum, 1.0 / NSAMP)
            msq = sb.tile([128, n], F32, name=f"bn_{name}_msq")
            nc.vector.tensor_scalar(msq[:], gsumsq, 1.0 / NSAMP, EPS,
                                    OP.mult, OP.add)
            var = sb.tile([128, n], F32, name=f"bn_{name}_var")
            nc.vector.tensor_tensor(var[:], mean[:], mean[:], OP.mult)
            nc.vector.tensor_tensor(var[:], msq[:], var[:], OP.subtract)
            rsd = sb.tile([128, n], F32, name=f"bn_{name}_rsd")
            if os.environ.get("BASS_BN_LNEXP") == "1":
                lg = sb.tile([128, n], F32, name=f"bn_{name}_lg")
                nc.scalar.activation(lg[:], var[:], AF.Ln, bias=zcol[:])
                nc.scalar.activation(rsd[:], lg[:], AF.Exp, bias=zcol[:],
                                     scale=-0.5)
            else:
                scr = sb.tile([128, n], I32, name=f"bn_{name}_scr")
                rsqrt_dve(rsd, var[:], scr, n, name)
            a = sb.tile([128, n], F32, name=f"bn_{name}_a")
            b = sb.tile([128, n], F32, name=f"bn_{name}_b")
            nc.vector.tensor_tensor(a[:], gamma, rsd[:], OP.mult)
            nc.vector.tensor_tensor(b[:], a[:], mean[:], OP.mult)
            nc.vector.tensor_tensor(b[:], beta, b[:], OP.subtract)
            return a, b

        for _rep in range(_REPS):
            xb_sb = sb.tile([128, 4, HW], BF16)
            for k in range(4):
                for hh in range(2):
                    nc.sync.dma_start(
                        xb_sb[:, k, hh * 512:(hh + 1) * 512],
                        xb_d[k * 128:(k + 1) * 128, hh * 512:(hh + 1) * 512])

            # ---------------- conv1 (512 -> 128) ----------------
            o1_ps = psum.tile([128, HW], F32, tag="A")
            for n in range(2):
                for k in range(4):
                    nc.tensor.matmul(
                        o1_ps[:, n * 512:(n + 1) * 512],
                        w1t_sb[:, k, :],
                        xb_sb[:, k, n * 512:(n + 1) * 512],
                        start=(k == 0), stop=(k == 3),
                    )

            s1 = sb.tile([128, 2], F32)
            nc.vector.tensor_reduce(s1[:, 0:1], o1_ps[:], AX.X, OP.add)
            sq1 = work.tile([128, HW], BF16, tag="sq", bufs=1)
            nc.scalar.activation(sq1[:], o1_ps[:], AF.Square, bias=zcol[:],
                                 accum_out=s1[:, 1:2])
            dump("o1", o1_ps[:])
            g1 = allreduce(s1[:], 2, "bn1")
            a1, b1 = bn_params(g1[:, 0:1], g1[:, 1:2],
                               gb12_sb[:, 0:1], gb12_sb[:, 1:2], "1")
            dump("a1", a1[:])
            dump("b1", b1[:])

            # x1 = relu(a1*o1 + b1), written into the zero-padded 38x38 field
            nc.scalar.activation(
                x1p[:, PAD:PAD + H, PAD:PAD + W],
                o1_ps[:].rearrange("p (y x) -> p y x", y=H),
                AF.Relu, bias=b1[:], scale=a1[:],
            )

            dump("x1p", x1p[:].rearrange("p y x -> p (y x)"))
            # ---------------- q/k/v grouped 1x1 convs ----------------

            col_splits = [(0, 512), (512, 1024), (1024, PHW)]

            def kv_conv(widx, name, ps_tag):
                ps = psum.tile([128, PHW], F32, tag=ps_tag, name=f"{name}_ps")
                x1p_flat = x1p[:].rearrange("p y x -> p (y x)")
                for (c0, c1) in col_splits:
                    nc.tensor.matmul(
                        ps[:, c0:c1],
                        mqkv_sb[:, widx, :],
                        x1p_flat[:, c0:c1],
                        start=True, stop=True,
                    )
                fld = sb.tile([128, PW, PW], BF16, name=f"{name}_fld")
                nc.scalar.activation(
                    fld[:], ps[:].rearrange("p (y x) -> p y x", y=PW),
                    AF.Copy,
                )
                odd = sb.tile([128, PHW - 1], BF16, name=f"{name}_odd")
                nc.vector.tensor_copy(
                    odd[:], fld[:].rearrange("p y x -> p (y x)")[:, 1:PHW])
                return fld, odd

            q_ps = psum.tile([128, HW], F32, tag="A")
            for n in range(2):
                nc.tensor.matmul(
                    q_ps[:, n * 512:(n + 1) * 512],
                    mqkv_sb[:, 0, :],
                    x1p[:, PAD + n * 16:PAD + (n + 1) * 16, PAD:PAD + W],
                    start=True, stop=True,
                )
            q_bf = sb.tile([128, HW], BF16)
            nc.scalar.activation(q_bf[:], q_ps[:], AF.Copy)
            dump("q", q_bf[:])
            k_fld, k_odd = kv_conv(1, "k", "B")
            dump("kf", k_fld[:].rearrange("p y x -> p (y x)"))
            v_fld, v_odd = kv_conv(2, "v", "C")

            # ---------------- attention over 49 shifts ----------------
            z_ps = psum.tile([128, HW], F32, tag="A")
            s_ps = psum.tile([128, HW], F32, tag="B")

            q_b7 = q_bf[:].rearrange("p (a h) -> p a h", a=1).to_broadcast(
                [128, KS, HW])

            # Software-pipelined: the e*v multiply and S accumulation of
            # iteration kh-1 are emitted AFTER exp(kh) is issued, so the DVE
            # never stalls waiting on the ACT exp it just fed.
            def emit_ev(kh, eb):
                evb = work.tile([128, KS, HW], BF16, tag="evb",
                                name=f"evb_{kh}")
                v_flat = v_fld[:].rearrange("p y x -> p (y x)")
                ev_even = _sview(v_flat, kh * PW, [(2, 4), (PW, H), (1, W)])
                ev_odd = _sview(v_odd[:], kh * PW, [(2, 3), (PW, H), (1, W)])
                pool_pos = sorted(_EV_POOL_KWS)
                nc.vector.tensor_tensor(
                    evb[:, 0:4, :].rearrange("p k (y x) -> p k y x", y=H),
                    eb[:, 0:4, :].rearrange("p k (y x) -> p k y x", y=H),
                    ev_even, OP.mult)
                if 6 in pool_pos:
                    nc.vector.tensor_tensor(
                        evb[:, 4:6, :].rearrange("p k (y x) -> p k y x", y=H),
                        eb[:, 4:6, :].rearrange("p k (y x) -> p k y x", y=H),
                        _sview(v_odd[:], kh * PW, [(2, 2), (PW, H), (1, W)]),
                        OP.mult)
                    nc.gpsimd.tensor_tensor(
                        evb[:, 6, :].rearrange("p (y x) -> p y x", y=H),
                        eb[:, 6, :].rearrange("p (y x) -> p y x", y=H),
                        _sview(v_odd[:], kh * PW + 4, [(PW, H), (1, W)]),
                        OP.mult)
                else:
                    nc.vector.tensor_tensor(
                        evb[:, 4:7, :].rearrange("p k (y x) -> p k y x", y=H),
                        eb[:, 4:7, :].rearrange("p k (y x) -> p k y x", y=H),
                        ev_odd, OP.mult)
                # S += sum_planes(ev)
                for pl in range(KS):
                    for hh in range(2):
                        nc.tensor.matmul(
                            s_ps[:, hh * 512:(hh + 1) * 512],
                            id_sb[:], evb[:, pl, hh * 512:(hh + 1) * 512],
                            start=(kh == 0 and pl == 0),
                            stop=(kh == KS - 1 and pl == KS - 1),
                            skip_group_check=True,
                        )

            prev = None
            for kh in range([0, KS][not _NO_ATT]):
                mb = work.tile([128, KS, HW], BF16, tag="mb")
                # m = k_shift + (rel + bk); rel is a per-partition scalar
                for pos, kw in enumerate(KW_ORDER):
                    kap = kh * KS + kw
                    if kw % 2 == 0:
                        view = k_fld[:, kh:kh + H, kw:kw + W]
                    else:
                        view = _sview(k_odd[:], kh * PW + kw - 1,
                                      [(PW, H), (1, W)])
                    dst = mb[:, pos, :].rearrange("p (y x) -> p y x", y=H)
                    if kw in ACT_ADD_KWS:
                        nc.scalar.activation(dst, view, AF.Identity,
                                             bias=relc_sb[:, kap:kap + 1])
                    elif kw in POOL_ADD_KWS:
                        nc.gpsimd.tensor_scalar_add(dst, view,
                                                    relc_sb[:, kap:kap + 1])
                    else:
                        nc.vector.tensor_scalar_add(dst, view,
                                                    relc_sb[:, kap:kap + 1])
                # l = m * q  (one batched bf16 2x tensor_tensor)
                nc.vector.tensor_tensor(mb[:], mb[:], q_b7, OP.mult)
                # e = exp(l)
                eb = work.tile([128, KS, HW], BF16, tag="eb", bufs=3,
                               name=f"eb_{kh}")
                nc.scalar.activation(eb[:], mb[:], AF.Exp, bias=zcol[:])
                # Z += sum_planes(e) on the TensorEngine
                for pl in range(KS):
                    for hh in range(2):
                        nc.tensor.matmul(
                            z_ps[:, hh * 512:(hh + 1) * 512],
                            id_sb[:], eb[:, pl, hh * 512:(hh + 1) * 512],
                            start=(kh == 0 and pl == 0),
                            stop=(kh == KS - 1 and pl == KS - 1),
                            skip_group_check=True,
                        )
                if kh == 0:
                    dump("eb0", eb[:].rearrange("p k h -> p (k h)"))
                if prev is not None:
                    emit_ev(*prev)
                prev = (kh, eb)
            if prev is not None:
                emit_ev(*prev)

            # att = S / Z, then BN2 + relu
            dump("z", z_ps[:])
            dump("s", s_ps[:])
            rz = sb.tile([128, HW], F32)
            nc.vector.reciprocal_approx_fast(rz[:], z_ps[:])
            att = sb.tile([128, HW], F32)
            if _ATT_ENG == "p":
                nc.gpsimd.tensor_tensor(att[:], s_ps[:], rz[:], OP.mult)
            else:
                nc.vector.tensor_tensor(att[:], s_ps[:], rz[:], OP.mult)

            dump("att", att[:])
            s2 = sb.tile([128, 2], F32)
            nc.vector.tensor_reduce(s2[:, 0:1], att[:], AX.X, OP.add)
            sq2 = work.tile([128, HW], BF16, tag="sq", bufs=1)
            nc.scalar.activation(sq2[:], att[:], AF.Square, bias=zcol[:],
                                 accum_out=s2[:, 1:2])
            g2 = allreduce(s2[:], 2, "bn2")
            a2, b2 = bn_params(g2[:, 0:1], g2[:, 1:2],
                               gb12_sb[:, 2:3], gb12_sb[:, 3:4], "2")
            x2 = sb.tile([128, HW], BF16)
            nc.scalar.activation(x2[:], att[:], AF.Relu, bias=b2[:], scale=a2[:])

            # ---------------- conv3 (128 -> 512) + BN3 + residual ----------------
            xf_sb = sb.tile([128, 4, HW], F32)
            for k in range(4):
                nc.sync.dma_start(xf_sb[:, k, :], xf_d[k * 128:(k + 1) * 128, :])
            o3_sb = sb.tile([128, 4, HW], F32)
            s3 = sb.tile([128, 8], F32)
            for j in range(4):
                o3_ps = psum.tile([128, HW], F32, tag=["C", "B"][j % 2],
                                  name=f"o3_ps_{j}")
                for n in range(2):
                    nc.tensor.matmul(
                        o3_ps[:, n * 512:(n + 1) * 512],
                        w3t_sb[:, j * 128:(j + 1) * 128],
                        x2[:, n * 512:(n + 1) * 512],
                        start=True, stop=True,
                    )
                sq3 = work.tile([128, HW], BF16, tag="sq", bufs=1,
                                name=f"sq3_{j}")
                nc.scalar.activation(sq3[:], o3_ps[:], AF.Square, bias=zcol[:],
                                     accum_out=s3[:, 2 * j + 1:2 * j + 2])
                if _O3_ENG == "p":
                    nc.gpsimd.tensor_scalar(o3_sb[:, j, :], o3_ps[:], 0.0, None,
                                            OP.add,
                                            accum_out=s3[:, 2 * j:2 * j + 1])
                else:
                    nc.scalar.activation(o3_sb[:, j, :], o3_ps[:], AF.Copy,
                                         accum_out=s3[:, 2 * j:2 * j + 1])

            dump("o3", o3_sb[:].rearrange("p j h -> p (j h)"))
            g3 = allreduce(s3[:], 8, "bn3")
            # batched BN3 params for all 4 chunks (compact copies first so the
            # rsqrt bitcast sees contiguous columns)
            gsum3 = sb.tile([128, 4], F32)
            nc.vector.tensor_copy(gsum3[:], g3[:, 0:8:2])
            gsq3 = sb.tile([128, 4], F32)
            nc.vector.tensor_copy(gsq3[:], g3[:, 1:8:2])
            a3, b3 = bn_params(gsum3[:], gsq3[:],
                               gb3_sb[:, 0:8:2], gb3_sb[:, 1:8:2], "3", n=4)
            for j in range(4):
                t3 = work.tile([128, HW], F32, tag="t3", name=f"t3_{j}")
                if _FIN_STT == "p":
                    nc.gpsimd.scalar_tensor_tensor(
                        t3[:], o3_sb[:, j, :], a3[:, j:j + 1], xf_sb[:, j, :],
                        OP.mult, OP.add)
                else:
                    nc.vector.scalar_tensor_tensor(
                        t3[:], o3_sb[:, j, :], a3[:, j:j + 1], xf_sb[:, j, :],
                        OP.mult, OP.add)
                ot = work.tile([128, HW], F32, tag="ot", name=f"ot_{j}")
                if _FIN_RELU == "p":
                    nc.gpsimd.tensor_scalar(ot[:], t3[:], b3[:, j:j + 1], 0.0,
                                            OP.add, OP.max)
                else:
                    nc.scalar.activation(ot[:], t3[:], AF.Relu,
                                         bias=b3[:, j:j + 1])
                nc.sync.dma_start(out_d[j * 128:(j + 1) * 128, :], ot[:])

    nc.compile()
    return nc


_NC = None


def _get_nc():
    global _NC
    if _NC is None:
        _NC = _build_nc()
    return _NC


def _prep_inputs(x, W1, g1, b1, Wq, Wk, bk, Wv, bv, rel_x, rel_y, g2, b2, W3, g3, b3):
    f32 = np.float32
    bf = ml_dtypes.bfloat16

    # channel permutation: new partition -> old channel within the 128 planes
    perm = np.zeros(PLANES, dtype=np.int64)
    for g in range(GROUPS):
        for d in range(D):
            p = g * REL + d if d < REL else 64 + g * REL + (d - REL)
            perm[p] = g * D + d

    W1p = np.ascontiguousarray(W1[perm, :])                     # [128, 512]
    w1t = np.ascontiguousarray(W1p.T).astype(bf)                # [512, 128]

    def block_mat(Wg):
        M = np.zeros((PLANES, PLANES), dtype=f32)
        for po in range(PLANES):
            g = (po % 64) // REL
            o = perm[po] - g * D
            for pi_d in range(D):
                pi = g * REL + pi_d if pi_d < REL else 64 + g * REL + (pi_d - REL)
                M[po, pi] = Wg[g, o, pi_d]
        return M

    mqkv = np.stack([np.ascontiguousarray(block_mat(Wg).T)
                     for Wg in (Wq, Wk, Wv)]).astype(bf)        # [3,128,128] (lhsT)

    # rel columns [128, 49]; k-conv bias bk is folded in (exact: the adds
    # apply rel+bk to every window position, matching conv-bias semantics
    # including the zero-padding border). v-conv bias bv is dropped: BN2 is
    # invariant to the per-channel shift it would produce.
    bkp = bk.reshape(-1)[perm].astype(f32)
    relc = np.zeros((PLANES, KS * KS), dtype=f32)
    for p in range(PLANES):
        g = (p % 64) // REL
        dd = perm[p] - g * D
        for kap in range(KS * KS):
            kh, kw = divmod(kap, KS)
            base = rel_x[dd, kh, 0] if dd < REL else rel_y[dd - REL, 0, kw]
            relc[p, kap] = base + bkp[p]

    gb12 = np.stack([g1[perm], b1[perm], g2[perm], b2[perm]], axis=1).astype(f32)

    W3p = np.ascontiguousarray(W3[:, perm])                     # [512, 128]
    w3t = np.ascontiguousarray(W3p.T).astype(bf)                # [128, 512]

    gb3 = np.zeros((PLANES, 8), dtype=f32)
    for j in range(4):
        gb3[:, 2 * j] = g3[j * 128:(j + 1) * 128]
        gb3[:, 2 * j + 1] = b3[j * 128:(j + 1) * 128]

    id128 = np.eye(PLANES, dtype=f32).astype(bf)

    shared = dict(w1t=w1t, mqkv=mqkv, w3t=w3t, relc=relc,
                  gb12=gb12, gb3=gb3, id128=id128)
    in_maps = []
    for c in range(N_CORES):
        xi = np.ascontiguousarray(x[c].reshape(C_IN, HW)).astype(f32)
        m = dict(shared)
        m["xf"] = xi
        m["xb"] = xi.astype(bf)
        in_maps.append(m)
    return in_maps


def _run(inputs, **kw):
    nc = _get_nc()
    in_maps = _prep_inputs(**inputs)
    res = run_bass_kernel_spmd(nc, in_maps, core_ids=list(range(N_CORES)), **kw)
    out = np.stack([res.results[c]["out"].reshape(C_IN, H, W)
                    for c in range(N_CORES)]).astype(np.float32)
    return out, res


def kernel(**inputs):
    out, _ = _run(inputs)
    return out


# revision 10
# speedup vs baseline: 2.3950x; 2.0303x over previous
# Trainium2 Bass kernel for nn_BottleNeck (sparse local attention bottleneck).
# Software-pipelined version: the tail of rep r-1 (BN2/conv3/BN3/residual) and
# the head of rep r+1 (conv1/BN1/x1p/qkv convs) are emitted at injection
# points inside rep r's attention loop, so in steady state every phase
# overlaps the attention of the adjacent rep and the DVE (the busiest engine)
# never waits on collectives or PSUM frees.
#
# PSUM banks: A = Z, B = S (attention accumulators, live for a whole rep);
# C = everything else (o1/q/k/v/o3) time-shared serially.
#
# See kernel header comments in the flat version for the math-level notes
# (channel permutation, bk folded into rel columns, bv dropped via BN2
# shift-invariance, DVE-only rsqrt for BN params => single ACT table set).

import os
from contextlib import ExitStack

import numpy as np
import ml_dtypes

import concourse.bass as bass
import concourse.mybir as mybir
import concourse.tile as tile
from concourse import bacc
from concourse.ap import AP
from concourse.bass_utils import run_bass_kernel_spmd

F32 = mybir.dt.float32
BF16 = mybir.dt.bfloat16
I32 = mybir.dt.int32
AF = mybir.ActivationFunctionType
OP = mybir.AluOpType
AX = mybir.AxisListType

B, C_IN, H, W = 8, 512, 32, 32
PLANES, GROUPS, KS, PAD = 128, 8, 7, 3
D = PLANES // GROUPS
REL = D // 2
HW = H * W
PW = W + 2 * PAD            # 38
PHW = PW * PW               # 1444
EPS = 1e-5
N_CORES = 8
NSAMP = float(B * HW)

KW_ORDER = [0, 2, 4, 6, 1, 3, 5]


def _kwset(name, default):
    v = os.environ.get(name, default)
    return {int(x) for x in v.split(",") if x != ""}

ACT_ADD_KWS = _kwset("BASS_ADD_A", "1")
POOL_ADD_KWS = _kwset("BASS_ADD_P", "")
_ATT_ENG = os.environ.get("BASS_ATT_ENG", "v")
_O3_ENG = os.environ.get("BASS_O3_ENG", "a")
_FIN_RELU = os.environ.get("BASS_FIN_RELU", "a")
_FIN_STT = os.environ.get("BASS_FIN_STT", "v")
_CC_MODE = os.environ.get("BASS_CC_MODE", "ar")
_NO_CC = os.environ.get("BASS_NO_CC") == "1"
_REPS = int(os.environ.get("BASS_REPS", "1"))

_RSQRT_K = 0x5F3759DF


def _sview(flat_ap, off, dims):
    return AP(flat_ap.tensor, off, [list(flat_ap.ap[0])] + [list(d) for d in dims])


def _build_nc():
    nc = bacc.Bacc("TRN2", target_bir_lowering=False, debug=False,
                   num_devices=N_CORES)

    xf_d = nc.dram_tensor("xf", [C_IN, HW], F32, kind="ExternalInput")
    xb_d = nc.dram_tensor("xb", [C_IN, HW], BF16, kind="ExternalInput")
    w1t_d = nc.dram_tensor("w1t", [C_IN, PLANES], BF16, kind="ExternalInput")
    mqkv_d = nc.dram_tensor("mqkv", [3, PLANES, PLANES], BF16, kind="ExternalInput")
    w3t_d = nc.dram_tensor("w3t", [PLANES, 4 * PLANES], BF16, kind="ExternalInput")
    relc_d = nc.dram_tensor("relc", [PLANES, KS * KS], F32, kind="ExternalInput")
    gb12_d = nc.dram_tensor("gb12", [PLANES, 4], F32, kind="ExternalInput")
    gb3_d = nc.dram_tensor("gb3", [PLANES, 8], F32, kind="ExternalInput")
    id_d = nc.dram_tensor("id128", [PLANES, PLANES], BF16, kind="ExternalInput")
    out_d = nc.dram_tensor("out", [C_IN, HW], F32, kind="ExternalOutput")

    with tile.TileContext(nc) as tc, ExitStack() as ctx:
        const = ctx.enter_context(tc.tile_pool(name="const", bufs=1))
        sb = ctx.enter_context(tc.tile_pool(name="sb", bufs=1))
        fld2 = ctx.enter_context(tc.tile_pool(name="fld2", bufs=2))
        work = ctx.enter_context(tc.tile_pool(name="work", bufs=2))
        psum = ctx.enter_context(tc.tile_pool(name="psum", bufs=1, space="PSUM"))
        dram = ctx.enter_context(tc.tile_pool(name="dram", bufs=1, space="DRAM"))

        # ---------------- constants / weights ----------------
        id_sb = const.tile([128, 128], BF16)
        nc.sync.dma_start(id_sb[:], id_d[:])
        w1t_sb = const.tile([128, 4, 128], BF16)
        for k in range(4):
            nc.sync.dma_start(w1t_sb[:, k, :], w1t_d[k * 128:(k + 1) * 128, :])
        mqkv_sb = const.tile([128, 3, 128], BF16)
        for i in range(3):
            nc.sync.dma_start(mqkv_sb[:, i, :], mqkv_d[i])
        w3t_sb = const.tile([128, 512], BF16)
        nc.sync.dma_start(w3t_sb[:], w3t_d[:])
        relc_sb = const.tile([128, 49], F32)
        nc.sync.dma_start(relc_sb[:], relc_d[:])
        gb12_sb = const.tile([128, 4], F32)
        nc.sync.dma_start(gb12_sb[:], gb12_d[:])
        gb3_sb = const.tile([128, 8], F32)
        nc.sync.dma_start(gb3_sb[:], gb3_d[:])

        zcol = const.tile([128, 1], F32)
        nc.gpsimd.memset(zcol[:], 0.0)
        expwarm = const.tile([128, 1], F32)
        nc.scalar.activation(expwarm[:], zcol[:], AF.Exp, bias=zcol[:])

        x1p = const.tile([128, PW, PW], BF16)
        nc.gpsimd.memset(x1p[:], 0.0)

        # ---------------- helpers ----------------
        def allreduce(src_ap, ncols, name):
            dst = sb.tile([128, ncols], F32, name=f"cc_{name}_res")
            if _NO_CC:
                nc.vector.tensor_scalar_mul(dst[:], src_ap, float(N_CORES))
                return dst
            cin = dram.tile([128, ncols], F32, name=f"cc_{name}_in")
            nc.sync.dma_start(cin[:], src_ap)
            cout = dram.tile([128, ncols], F32, name=f"cc_{name}_out")
            nc.gpsimd.collective_compute(
                "AllReduce", OP.add,
                replica_groups=[list(range(N_CORES))],
                ins=[cin[:].opt()], outs=[cout[:].opt()],
            )
            nc.sync.dma_start(dst[:], cout[:])
            return dst

        def rsqrt_dve(y_f32, x_ap, scr_i32, n, name):
            xi = x_ap.bitcast(I32)
            nc.vector.tensor_scalar(scr_i32[:], xi, 1, None,
                                    OP.logical_shift_right)
            nc.vector.tensor_scalar(scr_i32[:], scr_i32[:], -1, None,
                                    OP.bitwise_xor)
            nc.vector.tensor_scalar(scr_i32[:], scr_i32[:], _RSQRT_K + 1, None,
                                    OP.add)
            y0 = scr_i32[:].bitcast(F32)
            t = sb.tile([128, n], F32, name=f"rs_{name}_t")
            for it in range(2):
                src = y0 if it == 0 else y_f32[:]
                nc.vector.tensor_tensor(t[:], src, src, OP.mult)
                nc.vector.scalar_tensor_tensor(t[:], t[:], 0.5, x_ap,
                                               OP.mult, OP.mult)
                nc.vector.tensor_scalar(t[:], t[:], -1.0, 1.5, OP.mult, OP.add)
                nc.vector.tensor_tensor(y_f32[:], src, t[:], OP.mult)

        def bn_params(gsum, gsumsq, gamma, beta, name, n=1):
            mean = sb.tile([128, n], F32, name=f"bn_{name}_mean")
            nc.vector.tensor_scalar_mul(mean[:], gsum, 1.0 / NSAMP)
            msq = sb.tile([128, n], F32, name=f"bn_{name}_msq")
            nc.vector.tensor_scalar(msq[:], gsumsq, 1.0 / NSAMP, EPS,
                                    OP.mult, OP.add)
            var = sb.tile([128, n], F32, name=f"bn_{name}_var")
            nc.vector.tensor_tensor(var[:], mean[:], mean[:], OP.mult)
            nc.vector.tensor_tensor(var[:], msq[:], var[:], OP.subtract)
            rsd = sb.tile([128, n], F32, name=f"bn_{name}_rsd")
            scr = sb.tile([128, n], I32, name=f"bn_{name}_scr")
            rsqrt_dve(rsd, var[:], scr, n, name)
            a = sb.tile([128, n], F32, name=f"bn_{name}_a")
            b = sb.tile([128, n], F32, name=f"bn_{name}_b")
            nc.vector.tensor_tensor(a[:], gamma, rsd[:], OP.mult)
            nc.vector.tensor_tensor(b[:], a[:], mean[:], OP.mult)
            nc.vector.tensor_tensor(b[:], beta, b[:], OP.subtract)
            return a, b

        # ---------------- per-rep stage emitters ----------------
        # Stage functions take/extend a per-rep state dict `st`.

        def head_a(st):
            """xb load + conv1 + BN1 partial stats + AR1 launch."""
            r = st["r"]
            xb_sb = sb.tile([128, 4, HW], BF16, name="xb")
            for k in range(4):
                for hh in range(2):
                    nc.sync.dma_start(
                        xb_sb[:, k, hh * 512:(hh + 1) * 512],
                        xb_d[k * 128:(k + 1) * 128, hh * 512:(hh + 1) * 512])
            o1_ps = psum.tile([128, HW], F32, tag="C", name="o1_ps")
            for n in range(2):
                for k in range(4):
                    nc.tensor.matmul(
                        o1_ps[:, n * 512:(n + 1) * 512],
                        w1t_sb[:, k, :],
                        xb_sb[:, k, n * 512:(n + 1) * 512],
                        start=(k == 0), stop=(k == 3),
                    )
            s1 = sb.tile([128, 2], F32, name="s1")
            nc.vector.tensor_reduce(s1[:, 0:1], o1_ps[:], AX.X, OP.add)
            sq1 = work.tile([128, HW], BF16, tag="sq", bufs=1, name="sq1")
            nc.scalar.activation(sq1[:], o1_ps[:], AF.Square, bias=zcol[:],
                                 accum_out=s1[:, 1:2])
            st["o1_ps"] = o1_ps
            st["g1"] = allreduce(s1[:], 2, "bn1")

        def head_b(st):
            """BN1 params + x1p."""
            g1 = st["g1"]
            a1, b1 = bn_params(g1[:, 0:1], g1[:, 1:2],
                               gb12_sb[:, 0:1], gb12_sb[:, 1:2], "1")
            nc.scalar.activation(
                x1p[:, PAD:PAD + H, PAD:PAD + W],
                st["o1_ps"][:].rearrange("p (y x) -> p y x", y=H),
                AF.Relu, bias=b1[:], scale=a1[:],
            )

        def head_c(st):
            """q/k/v grouped convs + field copies."""
            col_splits = [(0, 512), (512, 1024), (1024, PHW)]
            x1p_flat = x1p[:].rearrange("p y x -> p (y x)")

            q_ps = psum.tile([128, HW], F32, tag="C", name="q_ps")
            for n in range(2):
                nc.tensor.matmul(
                    q_ps[:, n * 512:(n + 1) * 512],
                    mqkv_sb[:, 0, :],
                    x1p[:, PAD + n * 16:PAD + (n + 1) * 16, PAD:PAD + W],
                    start=True, stop=True,
                )
            q_bf = fld2.tile([128, HW], BF16, tag="q", name="q_bf")
            nc.scalar.activation(q_bf[:], q_ps[:], AF.Copy)

            def kv_conv(widx, name):
                ps = psum.tile([128, PHW], F32, tag="C", name=f"{name}_ps")
                for (c0, c1) in col_splits:
                    nc.tensor.matmul(
                        ps[:, c0:c1],
                        mqkv_sb[:, widx, :],
                        x1p_flat[:, c0:c1],
                        start=True, stop=True,
                    )
                fld = fld2.tile([128, PW, PW], BF16, tag=f"{name}f",
                                name=f"{name}_fld")
                nc.scalar.activation(
                    fld[:], ps[:].rearrange("p (y x) -> p y x", y=PW), AF.Copy)
                odd = fld2.tile([128, PHW - 1], BF16, tag=f"{name}o",
                                name=f"{name}_odd")
                nc.vector.tensor_copy(
                    odd[:], fld[:].rearrange("p y x -> p (y x)")[:, 1:PHW])
                return fld, odd

            st["q_bf"] = q_bf
            st["k_fld"], st["k_odd"] = kv_conv(1, "k")
            st["v_fld"], st["v_odd"] = kv_conv(2, "v")

        def att_div(st):
            """att = S * (1/Z); BN2 partial stats; AR2 launch. Frees A/B."""
            z_ps, s_ps = st["z_ps"], st["s_ps"]
            rz = sb.tile([128, HW], F32, name="rz")
            nc.vector.reciprocal_approx_fast(rz[:], z_ps[:])
            att = sb.tile([128, HW], F32, name="att")
            nc.vector.tensor_tensor(att[:], s_ps[:], rz[:], OP.mult)
            s2 = sb.tile([128, 2], F32, name="s2")
            nc.vector.tensor_reduce(s2[:, 0:1], att[:], AX.X, OP.add)
            sq2 = work.tile([128, HW], BF16, tag="sq", bufs=1, name="sq2")
            nc.scalar.activation(sq2[:], att[:], AF.Square, bias=zcol[:],
                                 accum_out=s2[:, 1:2])
            st["att"] = att
            st["g2"] = allreduce(s2[:], 2, "bn2")

        def tail_xf(st):
            xf_sb = sb.tile([128, 4, HW], F32, name="xf_sb")
            for k in range(4):
                nc.sync.dma_start(xf_sb[:, k, :], xf_d[k * 128:(k + 1) * 128, :])
            st["xf_sb"] = xf_sb

        def tail_x2(st):
            g2 = st["g2"]
            a2, b2 = bn_params(g2[:, 0:1], g2[:, 1:2],
                               gb12_sb[:, 2:3], gb12_sb[:, 3:4], "2")
            x2 = sb.tile([128, HW], BF16, name="x2")
            nc.scalar.activation(x2[:], st["att"][:], AF.Relu,
                                 bias=b2[:], scale=a2[:])
            st["x2"] = x2

        def tail_o3(st):
            x2 = st["x2"]
            o3_sb = sb.tile([128, 4, HW], F32, name="o3_sb")
            s3 = sb.tile([128, 8], F32, name="s3")
            for j in range(4):
                o3_ps = psum.tile([128, HW], F32, tag="C", name=f"o3_ps_{j}")
                for n in range(2):
                    nc.tensor.matmul(
                        o3_ps[:, n * 512:(n + 1) * 512],
                        w3t_sb[:, j * 128:(j + 1) * 128],
                        x2[:, n * 512:(n + 1) * 512],
                        start=True, stop=True,
                    )
                sq3 = work.tile([128, HW], BF16, tag="sq", bufs=1,
                                name=f"sq3_{j}")
                nc.scalar.activation(sq3[:], o3_ps[:], AF.Square, bias=zcol[:],
                                     accum_out=s3[:, 2 * j + 1:2 * j + 2])
                nc.scalar.activation(o3_sb[:, j, :], o3_ps[:], AF.Copy,
                                     accum_out=s3[:, 2 * j:2 * j + 1])
            st["o3_sb"] = o3_sb
            st["g3"] = allreduce(s3[:], 8, "bn3")

        def tail_fin(st):
            g3 = st["g3"]
            gsum3 = sb.tile([128, 4], F32, name="gsum3")
            nc.vector.tensor_copy(gsum3[:], g3[:, 0:8:2])
            gsq3 = sb.tile([128, 4], F32, name="gsq3")
            nc.vector.tensor_copy(gsq3[:], g3[:, 1:8:2])
            a3, b3 = bn_params(gsum3[:], gsq3[:],
                               gb3_sb[:, 0:8:2], gb3_sb[:, 1:8:2], "3", n=4)
            o3_sb, xf_sb = st["o3_sb"], st["xf_sb"]
            for j in range(4):
                t3 = work.tile([128, HW], F32, tag="t3", name=f"t3_{j}")
                if _FIN_STT == "p":
                    nc.gpsimd.scalar_tensor_tensor(
                        t3[:], o3_sb[:, j, :], a3[:, j:j + 1], xf_sb[:, j, :],
                        OP.mult, OP.add)
                else:
                    nc.vector.scalar_tensor_tensor(
                        t3[:], o3_sb[:, j, :], a3[:, j:j + 1], xf_sb[:, j, :],
                        OP.mult, OP.add)
                ot = work.tile([128, HW], F32, tag="ot", name=f"ot_{j}")
                if _FIN_RELU == "p":
                    nc.gpsimd.tensor_scalar(ot[:], t3[:], b3[:, j:j + 1], 0.0,
                                            OP.add, OP.max)
                else:
                    nc.scalar.activation(ot[:], t3[:], AF.Relu,
                                         bias=b3[:, j:j + 1])
                nc.sync.dma_start(out_d[j * 128:(j + 1) * 128, :], ot[:])

        # ---------------- the pipelined rep loop ----------------
        def attention(st, inject):
            """Emit rep st's attention; call inject[kh]() after each kh."""
            z_ps = psum.tile([128, HW], F32, tag="A", name="z_ps")
            s_ps = psum.tile([128, HW], F32, tag="B", name="s_ps")
            st["z_ps"], st["s_ps"] = z_ps, s_ps
            k_fld, k_odd = st["k_fld"], st["k_odd"]
            v_fld, v_odd = st["v_fld"], st["v_odd"]
            q_b7 = st["q_bf"][:].rearrange("p (a h) -> p a h", a=1) \
                .to_broadcast([128, KS, HW])

            def emit_ev(kh, eb):
                evb = work.tile([128, KS, HW], BF16, tag="evb",
                                name=f"evb_{kh}")
                v_flat = v_fld[:].rearrange("p y x -> p (y x)")
                ev_even = _sview(v_flat, kh * PW, [(2, 4), (PW, H), (1, W)])
                ev_odd = _sview(v_odd[:], kh * PW, [(2, 3), (PW, H), (1, W)])
                nc.vector.tensor_tensor(
                    evb[:, 0:4, :].rearrange("p k (y x) -> p k y x", y=H),
                    eb[:, 0:4, :].rearrange("p k (y x) -> p k y x", y=H),
                    ev_even, OP.mult)
                nc.vector.tensor_tensor(
                    evb[:, 4:7, :].rearrange("p k (y x) -> p k y x", y=H),
                    eb[:, 4:7, :].rearrange("p k (y x) -> p k y x", y=H),
                    ev_odd, OP.mult)
                for pl in range(KS):
                    for hh in range(2):
                        nc.tensor.matmul(
                            s_ps[:, hh * 512:(hh + 1) * 512],
                            id_sb[:], evb[:, pl, hh * 512:(hh + 1) * 512],
                            start=(kh == 0 and pl == 0),
                            stop=(kh == KS - 1 and pl == KS - 1),
                            skip_group_check=True,
                        )

            prev = None
            for kh in range(KS):
                mb = work.tile([128, KS, HW], BF16, tag="mb", name=f"mb_{kh}")
                for pos, kw in enumerate(KW_ORDER):
                    kap = kh * KS + kw
                    if kw % 2 == 0:
                        view = k_fld[:, kh:kh + H, kw:kw + W]
                    else:
                        view = _sview(k_odd[:], kh * PW + kw - 1,
                                      [(PW, H), (1, W)])
                    dst = mb[:, pos, :].rearrange("p (y x) -> p y x", y=H)
                    if kw in ACT_ADD_KWS:
                        nc.scalar.activation(dst, view, AF.Identity,
                                             bias=relc_sb[:, kap:kap + 1])
                    elif kw in POOL_ADD_KWS:
                        nc.gpsimd.tensor_scalar_add(dst, view,
                                                    relc_sb[:, kap:kap + 1])
                    else:
                        nc.vector.tensor_scalar_add(dst, view,
                                                    relc_sb[:, kap:kap + 1])
                nc.vector.tensor_tensor(mb[:], mb[:], q_b7, OP.mult)
                eb = work.tile([128, KS, HW], BF16, tag="eb", bufs=3,
                               name=f"eb_{kh}")
                nc.scalar.activation(eb[:], mb[:], AF.Exp, bias=zcol[:])
                for pl in range(KS):
                    for hh in range(2):
                        nc.tensor.matmul(
                            z_ps[:, hh * 512:(hh + 1) * 512],
                            id_sb[:], eb[:, pl, hh * 512:(hh + 1) * 512],
                            start=(kh == 0 and pl == 0),
                            stop=(kh == KS - 1 and pl == KS - 1),
                            skip_group_check=True,
                        )
                if prev is not None:
                    emit_ev(*prev)
                prev = (kh, eb)
                cb = inject.get(kh)
                if cb:
                    cb()
            emit_ev(*prev)

        reps = [{"r": r} for r in range(_REPS)]

        # prologue: head of rep 0
        head_a(reps[0])
        head_b(reps[0])
        head_c(reps[0])

        for r in range(_REPS):
            st = reps[r]
            pv = reps[r - 1] if r > 0 else None
            nx = reps[r + 1] if r + 1 < _REPS else None

            inject = {}
            if pv is not None:
                inject[0] = lambda pv=pv: tail_xf(pv)
                inject[1] = lambda pv=pv: tail_x2(pv)
                inject[2] = lambda pv=pv: tail_o3(pv)
                inject[4] = lambda pv=pv: tail_fin(pv)
            if nx is not None:
                inject[3] = lambda nx=nx: head_a(nx)
                inject[5] = lambda nx=nx: head_b(nx)
                # head_c of nx is emitted right after this rep's attention
                # (needs PSUM C after tail_o3(pv) and x1p after head_b).

            attention(st, inject)
            att_div(st)
            if nx is not None:
                head_c(nx)

        # epilogue: tail of the last rep
        last = reps[-1]
        tail_xf(last)
        tail_x2(last)
        tail_o3(last)
        tail_fin(last)

    nc.compile()
    return nc


_NC = None


def _get_nc():
    global _NC
    if _NC is None:
        _NC = _build_nc()
    return _NC


def _prep_inputs(x, W1, g1, b1, Wq, Wk, bk, Wv, bv, rel_x, rel_y, g2, b2, W3, g3, b3):
    f32 = np.float32
    bf = ml_dtypes.bfloat16

    perm = np.zeros(PLANES, dtype=np.int64)
    for g in range(GROUPS):
        for d in range(D):
            p = g * REL + d if d < REL else 64 + g * REL + (d - REL)
            perm[p] = g * D + d

    W1p = np.ascontiguousarray(W1[perm, :])
    w1t = np.ascontiguousarray(W1p.T).astype(bf)

    def block_mat(Wg):
        M = np.zeros((PLANES, PLANES), dtype=f32)
        for po in range(PLANES):
            g = (po % 64) // REL
            o = perm[po] - g * D
            for pi_d in range(D):
                pi = g * REL + pi_d if pi_d < REL else 64 + g * REL + (pi_d - REL)
                M[po, pi] = Wg[g, o, pi_d]
        return M

    mqkv = np.stack([np.ascontiguousarray(block_mat(Wg).T)
                     for Wg in (Wq, Wk, Wv)]).astype(bf)

    bkp = bk.reshape(-1)[perm].astype(f32)
    relc = np.zeros((PLANES, KS * KS), dtype=f32)
    for p in range(PLANES):
        g = (p % 64) // REL
        dd = perm[p] - g * D
        for kap in range(KS * KS):
            kh, kw = divmod(kap, KS)
            base = rel_x[dd, kh, 0] if dd < REL else rel_y[dd - REL, 0, kw]
            relc[p, kap] = base + bkp[p]

    gb12 = np.stack([g1[perm], b1[perm], g2[perm], b2[perm]], axis=1).astype(f32)

    W3p = np.ascontiguousarray(W3[:, perm])
    w3t = np.ascontiguousarray(W3p.T).astype(bf)

    gb3 = np.zeros((PLANES, 8), dtype=f32)
    for j in range(4):
        gb3[:, 2 * j] = g3[j * 128:(j + 1) * 128]
        gb3[:, 2 * j + 1] = b3[j * 128:(j + 1) * 128]

    id128 = np.eye(PLANES, dtype=f32).astype(bf)

    shared = dict(w1t=w1t, mqkv=mqkv, w3t=w3t, relc=relc,
                  gb12=gb12, gb3=gb3, id128=id128)
    in_maps = []
    for c in range(N_CORES):
        xi = np.ascontiguousarray(x[c].reshape(C_IN, HW)).astype(f32)
        m = dict(shared)
        m["xf"] = xi
        m["xb"] = xi.astype(bf)
        in_maps.append(m)
    return in_maps


def _run(inputs, **kw):
    nc = _get_nc()
    in_maps = _prep_inputs(**inputs)
    res = run_bass_kernel_spmd(nc, in_maps, core_ids=list(range(N_CORES)), **kw)
    out = np.stack([res.results[c]["out"].reshape(C_IN, H, W)
                    for c in range(N_CORES)]).astype(np.float32)
    return out, res


def kernel(**inputs):
    out, _ = _run(inputs)
    return out
